# revision 11
# baseline (speedup 1.0000x reference)
"""Trainium2 Bass kernel for nn_CoulombPotential (PhysNet-attenuated Coulomb energy).

Algorithm
---------
  per_system[s] = KE * sum_{pairs p: i<j, sys(i)=s} q[i] q[j] chi(d_p)
  chi(d) = phi(2d)/sqrt(d^2+1) + (1-phi(2d))/d,  phi = PhysNet switching fn.

Key structure (v2):
  * phi(2d) == 0 exactly for d >= 0.5, so pairs split into region A (d<0.5,
    full chi pipeline, ~37.5%) and region B (d>=0.5, chi = 1/d, ~62.5%).
  * streams are fp16 (d, q_i, q_j gathered on host = pure data movement);
    all arithmetic on device.  DVE runs 2-byte ops in 2x/4x perf modes.
  * 1/d = Ars(Square(d)) and 1/sqrt(d^2+1) = Ars(Square(d)+1) with
    Ars = Abs_reciprocal_sqrt -- Square/Ars/Copy share ONE activation table,
    so the ACT engine never swaps tables.
  * phi-polynomial core runs as one fused 8-stage custom DVE op
    u = ((192d-240)d+80)d^3 * a; since a = rs - r < 0 always,
    phi*a = relu(1-poly)*a = min(a - u, 0)  (cheap 4x tensor_scalar min).
  * per-row (512-slot) sums via tensor_scalar accum_out (4x mode),
    rows->systems via 0/1-selector matmuls on the PE; selectors are built
    on device from an iota constant + row->system ids (is_equal, 4x).
  * host only: mask/gather/sort/pad/cast + final scatter of 8x[128] outputs.
"""
import functools

import numpy as np

import concourse.bacc as bacc
import concourse.bass_utils as bass_utils
import concourse.mybir as mybir
import concourse.tile as tile

F32 = mybir.dt.float32
F16 = mybir.dt.float16
AF = mybir.ActivationFunctionType
OP = mybir.AluOpType

KE = 138.96
N_CORES = 8
S_TOTAL = 1024
SYS_PER_CORE = S_TOTAL // N_CORES  # 128

PART = 128
ROW = 512            # slots per logical row (per (system,branch) padding unit)
SUB_A = 7            # A-region sub-rows per partition  -> W_A = 3584 cols
SUB_B = 11           # B-region sub-rows per partition  -> W_B = 5632 cols
W_A = SUB_A * ROW
W_B = SUB_B * ROW
R_A = PART * SUB_A   # 896 A rows per core
R_B = PART * SUB_B   # 1408 B rows per core
NG = SUB_A + SUB_B   # reduce/selector groups (one per sub-row index)

# chunking for pipeline overlap (columns, 512-multiples)
A_CHUNKS = (1024, 1280, 1280)
B_CHUNKS = (2048, 2560, 1024)


def _a_slices():
    out, off = [], 0
    for w in A_CHUNKS:
        out.append(slice(off, off + w))
        off += w
    return out


@functools.lru_cache(maxsize=1)
def _register_polymul_op():
    """Fused DVE op: out = (((192*Src0 - 240)*Src0 + 80) * Src0^3) * Src1.
    (s0=192, s1=-240, imm2=80 at the call site.)"""
    import concourse.dve_ops as dve_ops
    from concourse.dve_spec import Spec, Src0, Src1, sq, lower
    from concourse.dve_uop import DveOpSpec
    for o in dve_ops.OPS:
        if o.name == "POLY_COULOMB_MUL":
            return o
    body = (((Src0 * dve_ops.C0 + dve_ops.C1) * Src0 + dve_ops.C2)
            * (sq(Src0) * Src0)) * Src1
    spec = Spec(body=body,
                reference=lambda in0, in1, s0, s1, imm2:
                    ((((in0 * s0 + s1) * in0 + imm2) * in0**3) * in1
                     ).astype(np.float32))
    shas = {v: DveOpSpec(name="POLY_COULOMB_MUL", opcode=1,
                         uops=lower(spec, ver=v)).sha(v) for v in ("v3", "v4")}
    op = dve_ops.DveOp("POLY_COULOMB_MUL", spec, subdim=False, uops_sha=shas)
    dve_ops.OPS.append(op)
    dve_ops.CUSTOM_DVE_SPECS[op.name] = op.spec
    dve_ops._SUB_OPCODE_FOR_NAME[op.name] = (
        dve_ops._CUSTOM_DVE_ROW_BASE + len(dve_ops.OPS) - 1)
    return op


def _act_raw(nc, out, in_, func, bias=0.0, scale=1.0):
    """Emit InstActivation directly (mirrors BassScalarEngine.activation
    lowering for Copy/Reciprocal-class funcs where bias stays immediate)."""
    eng = nc.scalar
    ins = [eng.lower_ap(in_)]
    for arg in (bias, scale, 0.0):
        ins.append(mybir.ImmediateValue(dtype=mybir.dt.float32,
                                        value=float(arg)))
    return eng.add_instruction(mybir.InstActivation(
        name=eng.bass.get_next_instruction_name(), func=func,
        ins=ins, outs=[eng.lower_ap(out)]))


@functools.lru_cache(maxsize=2)
def _build_nc(repeat=0):
    """repeat=0: straight-line kernel.  repeat=R>0: wrap the body in a
    hardware For_i loop running it R times (used for slope timing)."""
    poly_op = _register_polymul_op()
    nc = bacc.Bacc("TRN2", target_bir_lowering=False, debug=False,
                   enable_asserts=False, num_devices=N_CORES)
    d_a = nc.dram_tensor("d_a", [PART, W_A], F16, kind="ExternalInput")
    qi_a = nc.dram_tensor("qi_a", [PART, W_A], F16, kind="ExternalInput")
    qj_a = nc.dram_tensor("qj_a", [PART, W_A], F16, kind="ExternalInput")
    d_b = nc.dram_tensor("d_b", [PART, W_B], F16, kind="ExternalInput")
    qi_b = nc.dram_tensor("qi_b", [PART, W_B], F16, kind="ExternalInput")
    qj_b = nc.dram_tensor("qj_b", [PART, W_B], F16, kind="ExternalInput")
    rsys_in = nc.dram_tensor("rsys_in", [PART, NG], F32, kind="ExternalInput")
    iota_in = nc.dram_tensor("iota_in", [PART, PART], F16, kind="ExternalInput")
    out = nc.dram_tensor("out", [PART, 1], F32, kind="ExternalOutput")

    with tile.TileContext(nc) as tc:
        with (
            tc.tile_pool(name="io", bufs=2) as io,
            tc.tile_pool(name="wk", bufs=1) as wk,
            tc.tile_pool(name="wk2", bufs=2) as wk2,
            tc.tile_pool(name="acc", bufs=1) as acc,
            tc.tile_pool(name="psum", bufs=1, space="PSUM") as psp,
        ):
            ps = psp.tile([PART, 1], F32)

            def body():
                # ---- tiles ----
                dB = io.tile([PART, W_B], F16, tag="dB")
                qiB = io.tile([PART, W_B], F16, tag="qiB")
                qjB = io.tile([PART, W_B], F16, tag="qjB")
                dA = io.tile([PART, W_A], F16, tag="dA")
                qiA = io.tile([PART, W_A], F16, tag="qiA")
                qjA = io.tile([PART, W_A], F16, tag="qjA")
                rsys = io.tile([PART, NG], F32, tag="rsys")
                iota = io.tile([PART, PART], F16, tag="iota")

                qqA = wk2.tile([PART, W_A], F16, tag="qqA")
                s2A = wk.tile([PART, W_A], F16, tag="s2A")
                rsA = wk.tile([PART, W_A], F16, tag="rsA")
                rA = wk.tile([PART, W_A], F16, tag="rA")
                qqB = wk2.tile([PART, W_B], F16, tag="qqB")
                rB = wk2.tile([PART, W_B], F16, tag="rB")
                rsum = wk.tile([PART, NG], F32, tag="rsum")
                rsum16 = wk.tile([PART, NG], F16, tag="rsum16")
                red = wk.tile([PART, ROW], F16, tag="red")

                # ---- DMAs: A first (feeds ACT/DVE chains), B interleaved so
                # Pool qq chunks and late ACT recip work start ASAP; dB last
                # (only needed by the final recip passes).
                nc.sync.dma_start(rsys[:], rsys_in[:, :])
                nc.sync.dma_start(iota[:], iota_in[:, :])
                for cs in _a_slices():
                    nc.sync.dma_start(dA[:, cs], d_a[:, cs])
                nc.sync.dma_start(qiA[:], qi_a[:, :])
                nc.sync.dma_start(qjA[:], qj_a[:, :])
                boff = 0
                for bw in B_CHUNKS:
                    bs = slice(boff, boff + bw)
                    nc.sync.dma_start(qiB[:, bs], qi_b[:, bs])
                    nc.sync.dma_start(qjB[:, bs], qj_b[:, bs])
                    nc.sync.dma_start(dB[:, bs], d_b[:, bs])
                    boff += bw

                # ---- DVE: selectors first (only needs iota+rsys) ----
                sels = []
                for g in range(NG):
                    sel = io.tile([PART, PART], F16, tag=f"sel{g}")
                    nc.vector.tensor_scalar(sel[:], iota[:],
                                            rsys[:, g:g + 1], None,
                                            OP.is_equal)
                    sels.append(sel)

                # ---- Pool: all qq products (early-arriving inputs) ----
                nc.gpsimd.tensor_tensor(qqA[:], qiA[:], qjA[:], OP.mult)
                boff = 0
                for k, bw in enumerate(B_CHUNKS):
                    bs = slice(boff, boff + bw)
                    eng = nc.vector if k == len(B_CHUNKS) - 1 else nc.gpsimd
                    eng.tensor_tensor(qqB[:, bs], qiB[:, bs],
                                      qjB[:, bs], OP.mult)
                    boff += bw

                # ---- ACT: square/ars (one set), then all recips (other set)
                # => exactly 2 table swaps per iteration.
                for cs in _a_slices():
                    nc.scalar.activation(s2A[:, cs], dA[:, cs], AF.Square)
                    nc.scalar.activation(rsA[:, cs], s2A[:, cs],
                                         AF.Abs_reciprocal_sqrt, bias=1.0)
                    nc.scalar.activation(rA[:, cs], s2A[:, cs],
                                         AF.Abs_reciprocal_sqrt)
                boff = 0
                for bw in B_CHUNKS:
                    bs = slice(boff, boff + bw)
                    _act_raw(nc, rB[:, bs], dB[:, bs], AF.Reciprocal)
                    boff += bw

                # ---- DVE: A chain per chunk, then e/reduce work ----
                for cs in _a_slices():
                    nc.vector.tensor_tensor(rsA[:, cs], rsA[:, cs], rA[:, cs],
                                            OP.subtract)       # a  (in rsA)
                    nc.vector._custom_dve(poly_op, out=s2A[:, cs],
                                          in0=dA[:, cs], in1=rsA[:, cs],
                                          s0=192.0, s1=-240.0, imm2=80.0)
                    nc.vector.tensor_tensor(rsA[:, cs], rsA[:, cs],
                                            s2A[:, cs], OP.subtract)  # b=a-u
                    nc.vector.tensor_tensor(rA[:, cs], rsA[:, cs], rA[:, cs],
                                            OP.add)            # chi (in rA)
                    nc.vector.tensor_tensor(qqA[:, cs], qqA[:, cs], rA[:, cs],
                                            OP.mult)           # eA (in qqA)
                # A reduces + their matmuls first (data ready earliest)
                for g in range(SUB_A):
                    nc.vector.tensor_scalar(
                        red[:], qqA[:, g * ROW:(g + 1) * ROW], 1.0, 0.0,
                        OP.mult, OP.add, accum_out=rsum[:, g:g + 1])
                nc.vector.tensor_copy(rsum16[:, :SUB_A], rsum[:, :SUB_A])
                for g in range(SUB_A):
                    nc.tensor.matmul(ps[:], sels[g][:], rsum16[:, g:g + 1],
                                     start=(g == 0), stop=False)
                # B per chunk: e-mult, its reduce groups, copy, matmuls
                boff = 0
                g0 = SUB_A
                for k, bw in enumerate(B_CHUNKS):
                    bs = slice(boff, boff + bw)
                    nc.vector.tensor_tensor(qqB[:, bs], qqB[:, bs],
                                            rB[:, bs], OP.mult)  # eB
                    ng = bw // ROW
                    for j in range(ng):
                        g = g0 + j
                        c0 = boff + j * ROW
                        nc.vector.tensor_scalar(
                            red[:], qqB[:, c0:c0 + ROW], 1.0, 0.0,
                            OP.mult, OP.add,
                            accum_out=rsum[:, g:g + 1])
                    nc.vector.tensor_copy(rsum16[:, g0:g0 + ng],
                                          rsum[:, g0:g0 + ng])
                    for j in range(ng):
                        g = g0 + j
                        nc.tensor.matmul(ps[:], sels[g][:],
                                         rsum16[:, g:g + 1], start=False,
                                         stop=(g == NG - 1))
                    g0 += ng
                    boff += bw


            if repeat > 0:
                with tc.For_i(0, repeat, 1):
                    body()
            elif repeat < 0:
                for _ in range(-repeat):
                    body()
            else:
                body()
            res = acc.tile([PART, 1], F32, tag="res")
            nc.vector.tensor_scalar(res[:], ps[:], float(KE), None, OP.mult)
            nc.sync.dma_start(out[:], res[:])
    nc.compile()
    return nc


def _host_marshal(electrostatic_pair_indices, electrostatic_d_ij,
                  per_atom_charge, atomic_subsystem_indices):
    idx_i = np.asarray(electrostatic_pair_indices[0])
    idx_j = np.asarray(electrostatic_pair_indices[1])
    d = np.asarray(electrostatic_d_ij)[:, 0]
    q = np.asarray(per_atom_charge)[:, 0].astype(np.float32)
    sys_idx = np.asarray(atomic_subsystem_indices)

    keep = idx_i < idx_j
    ii = idx_i[keep]
    jj = idx_j[keep]
    dd = d[keep].astype(np.float32)
    seg = sys_idx[ii].astype(np.int64)
    br = (dd >= 0.5).astype(np.int64)        # 0 = region A, 1 = region B

    # sort by (system, branch); stable keeps determinism
    order = np.argsort(seg * 2 + br, kind="stable")
    ii = ii[order]
    jj = jj[order]
    dd = dd[order]
    seg = seg[order]
    br = br[order]

    key = seg * 2 + br                        # run id in [0, 2*S)
    counts2 = np.bincount(key, minlength=2 * S_TOTAL)  # per (sys,branch)
    run_start = np.concatenate([[0], np.cumsum(counts2)])
    counts = counts2[0::2] + counts2[1::2]    # per system

    # serpentine-assign systems to cores balanced by kept-pair count
    order_sys = np.argsort(-counts, kind="stable")
    k = np.arange(S_TOTAL)
    block, within = k // N_CORES, k % N_CORES
    core_of_rank = np.where(block % 2 == 0, within, N_CORES - 1 - within)
    sys_to_core = np.empty(S_TOTAL, np.int64)
    sys_to_core[order_sys] = core_of_rank
    sys_to_local = np.empty(S_TOTAL, np.int64)
    core_systems = np.empty((N_CORES, SYS_PER_CORE), np.int64)
    for c in range(N_CORES):
        mine = order_sys[core_of_rank == c]
        core_systems[c] = mine
        sys_to_local[mine] = np.arange(SYS_PER_CORE)

    # per-core, per-region row layout: each (sys,branch) run padded to rows
    rows_of_run = -(-counts2 // ROW)          # [2*S]
    run_row_base = np.empty(2 * S_TOTAL, np.int64)
    n_rows = np.zeros((N_CORES, 2), np.int64)
    for c in range(N_CORES):
        mine = core_systems[c]
        for b, r_cap in ((0, R_A), (1, R_B)):
            runs = mine * 2 + b
            rb = np.concatenate([[0], np.cumsum(rows_of_run[runs])])
            run_row_base[runs] = rb[:-1]
            n_rows[c, b] = rb[-1]
            assert rb[-1] <= r_cap, (c, b, rb[-1], r_cap)

    dest_core = sys_to_core[seg]
    # slot within the run, then within the region
    slot_in_run = np.arange(len(seg)) - run_start[key]
    dest_row = run_row_base[key] + slot_in_run // ROW
    dest_off = slot_in_run % ROW

    # row -> (partition, sub-row) -> flat stream index
    sub_of = np.where(br == 0, SUB_A, SUB_B)
    p_of = dest_row // sub_of
    n_of = dest_row % sub_of
    flat = p_of * np.where(br == 0, W_A, W_B) + n_of * ROW + dest_off

    qi_v = q[ii].astype(np.float16)
    qj_v = q[jj].astype(np.float16)
    dd16 = dd.astype(np.float16)

    iota_const = np.tile(np.arange(PART, dtype=np.float16), (PART, 1))

    in_maps = []
    for c in range(N_CORES):
        m = {}
        for b, W, sub, nm in ((0, W_A, SUB_A, "a"), (1, W_B, SUB_B, "b")):
            selp = (dest_core == c) & (br == b)
            f = flat[selp]
            ds = np.ones(PART * W, np.float16)
            qis = np.zeros(PART * W, np.float16)
            qjs = np.zeros(PART * W, np.float16)
            ds[f] = dd16[selp]
            qis[f] = qi_v[selp]
            qjs[f] = qj_v[selp]
            m["d_" + nm] = ds.reshape(PART, W)
            m["qi_" + nm] = qis.reshape(PART, W)
            m["qj_" + nm] = qjs.reshape(PART, W)

        # row -> local system map, per (partition, group)
        rsys = np.zeros((PART, NG), np.float32)
        for b, sub, r_cap, g0 in ((0, SUB_A, R_A, 0), (1, SUB_B, R_B, SUB_A)):
            row_sys = np.zeros(r_cap, np.int64)
            mine = core_systems[c]
            runs = mine * 2 + b
            nrows_runs = rows_of_run[runs]
            row_sys[:n_rows[c, b]] = np.repeat(sys_to_local[mine], nrows_runs)
            # row = p*sub + n  ->  rsys[p, g0+n]
            rs2 = row_sys.reshape(PART, sub)
            rsys[:, g0:g0 + sub] = rs2.astype(np.float32)
        m["rsys_in"] = rsys
        m["iota_in"] = iota_const
        in_maps.append(m)
    return in_maps, core_systems


def kernel(electrostatic_pair_indices, electrostatic_d_ij, per_atom_charge,
           atomic_subsystem_indices, num_systems):
    assert int(num_systems) == S_TOTAL
    in_maps, core_systems = _host_marshal(
        electrostatic_pair_indices, electrostatic_d_ij,
        per_atom_charge, atomic_subsystem_indices)
    nc = _build_nc()
    res = bass_utils.run_bass_kernel_spmd(nc, in_maps,
                                          core_ids=list(range(N_CORES)))
    full = np.empty(S_TOTAL, np.float32)
    for c in range(N_CORES):
        full[core_systems[c]] = res.results[c]["out"][:, 0]
    return full[:, None]
